# revision 13
# baseline (speedup 1.0000x reference)
"""GRU (hidden_size=1) kernel for Trainium2, data-parallel over batch on 8 cores.

v2: bf16 production + halo-block Jacobi scan.

Per core (B_loc = 256):
  - host stages x as bf16 xt[T, D, B_loc]; w_ih as bf16 wT[D, 3].
  - production in 16 chunks of 8 timesteps: w-stationary bf16 matmuls
    (4 contraction chunks, N=512) -> git_ps[3, 1024] f32 in PSUM, bias-add
    copy to bf16 git_sb (alternating ACT/DVE to balance engines), 16 PE
    transposes [3,128]->[128,3] per chunk into gib PSUM, DVE flush to f32
    gi planes gi_sb[128, 6T] (col = 6t + 3h + g).
  - scan via block-Jacobi fixed-point iteration: independent blocks
    (s0, L, H, K) covering [s0-H, s0+L); entry h=0 absorbed by the halo H
    (GRU forgets at ~e^-0.74/step); K sweeps of wide data-parallel ops:
      r,z = sigmoid(gi_rz + w01*hlag); n = tanh(gi_n + r*(w2*hlag+b2))
      h   = n*(1-z) + z*hlag
    per sweep: 6 DVE + 5 Pool + 2 ACT ops on [128, 2W] tiles. w_hh/b_hh2
    are compile-time immediates (rebuilt per weight values).
  - PE clock: dummy-matmul burst at start + 1 per chunk keeps PE_HAM at
    2.4 GHz (otherwise fp32r/bf16 matmuls run at 1.2 GHz).
  - partition-sum of HOUT via ones-matmul; host sums cores / divides by B.
"""

import numpy as np

import concourse.bass as bass
import concourse.mybir as mybir
from concourse.bass_types import AP
from concourse.tile import TileContext
from concourse.bass_utils import run_bass_kernel_spmd

F32 = mybir.dt.float32
BF16 = mybir.dt.bfloat16
AF = mybir.ActivationFunctionType
ALU = mybir.AluOpType

N_CORES = 8
B, T, D = 2048, 128, 512
B_LOC = B // N_CORES          # 256
NH = B_LOC // 128             # 2 column halves
NCH = D // 128                # 4 contraction chunks
TPC = 8                       # timesteps per chunk
NCHUNK = T // TPC             # 16
# Jacobi blocks: (s0, L, H, K); block covers t in [s0-H, s0+L), keeps
# [s0, s0+L). Entry h for halo blocks is 0; block 0 uses the real h0.
BLOCKS = [
    (0, 20, 0, 12),
    (20, 24, 12, 10),
    (44, 32, 12, 10),
    (76, 32, 12, 10),
    (108, 12, 12, 8),
    (120, 8, 12, 7),
]
assert sum(b[1] for b in BLOCKS) == T

_CACHE = {}


def build_nc(w0, w1, w2, bh2):
    nc = bass.Bass(trn_type="TRN2")

    xt = nc.dram_tensor("xt", [NCHUNK, D, TPC, B_LOC], BF16, kind="ExternalInput")
    wT = nc.dram_tensor("wT", [D, 3], BF16, kind="ExternalInput")
    cst = nc.dram_tensor("cst", [128, 4], F32, kind="ExternalInput")
    idb = nc.dram_tensor("idb", [3, 4], BF16, kind="ExternalInput")
    out = nc.dram_tensor("out", [1, 2 * T], F32, kind="ExternalOutput")

    with TileContext(nc) as tc:
        with (
            tc.tile_pool(name="xpool", bufs=4) as xpool,
            tc.tile_pool(name="consts", bufs=1) as consts,
            tc.tile_pool(name="gits", bufs=4) as gits,
            tc.tile_pool(name="scan", bufs=1) as scan,
            tc.tile_pool(name="hbuf", bufs=3) as hbp,
            tc.tile_pool(name="swp", bufs=3) as swp,
            tc.tile_pool(name="gtp", bufs=2, space="PSUM") as gtp,
            tc.tile_pool(name="gbp", bufs=2, space="PSUM") as gbp,
        ):
            # ---- x DMA helper ----
            def dma_chunk(c):
                x_sb = xpool.tile([128, NCH, TPC, B_LOC], BF16, name="x_sb")
                src = AP(
                    tensor=xt,
                    offset=c * TPC * D * B_LOC,
                    ap=[
                        [NCH * TPC * B_LOC, 128],  # partition p -> d = 4p + cc
                        [TPC * B_LOC, NCH],        # cc
                        [1, TPC * B_LOC],          # (t, b) contiguous 4KB
                    ],
                )
                nc.sync.dma_start(out=x_sb, in_=src)
                return x_sb

            # ---- constants (before the big x reads: tiny, unblock PE) ----
            wT_sb = consts.tile([128, NCH, 3], BF16)
            nc.sync.dma_start(
                out=wT_sb, in_=wT[:].rearrange("(p c) g -> p c g", p=128)
            )
            cst_sb = consts.tile([128, 4], F32)
            nc.sync.dma_start(out=cst_sb, in_=cst[:])
            idb_sb = consts.tile([3, 4], BF16)
            nc.sync.dma_start(out=idb_sb, in_=idb[:])
            id3 = idb_sb[:, 0:3]
            bias3 = cst_sb[0:3, 2:3]
            h0_sb = cst_sb[:, 0:2]
            ones_sb = consts.tile([128, 1], F32)
            nc.vector.memset(ones_sb, 1.0)

            x_tiles = {}
            for c in range(3):
                x_tiles[c] = dma_chunk(c)

            # PE warm-up burst: ~16 dense dummy matmuls on uninitialized
            # tiles keep PE_HAM's activity window busy through startup so
            # the real matmuls run at 2.4 GHz.
            dum_w = consts.tile([128, 1], BF16)
            dum_x = consts.tile([128, 512], BF16)
            nc.vector.memset(dum_w, 0.0)
            nc.gpsimd.memset(dum_x, 0.0)
            dum_ps = gbp.tile([1, 512], F32, tag="dum", name="dum_ps")
            for _ in range(16):
                nc.tensor.matmul(dum_ps, dum_w, dum_x, start=True, stop=True)

            # warm-up consumers of const DMAs (absorb semaphores)
            warm_sb = consts.tile([3, 1], F32)
            nc.scalar.copy(warm_sb, cst_sb[0:3, 3:4])
            warm_dv = consts.tile([3, 1], F32)
            nc.vector.tensor_copy(warm_dv, cst_sb[0:3, 3:4])
            warm_pl = consts.tile([3, 1], BF16)
            nc.gpsimd.tensor_copy(warm_pl, idb_sb[:, 3:4])
            warm_tp = gbp.tile([128, 4 * TPC * NH], BF16, tag="gib",
                               name="warm_tp")
            nc.tensor.transpose(warm_tp[0:4, 0:3], idb_sb[:, 0:4], id3)

            # ---- persistent buffers ----
            gi_sb = scan.tile([128, T * 6], F32)     # col = 6t + 3h + g
            hout = scan.tile([128, 2 * T], F32)      # col = 2t + h

            def gi_view(a0, W, g):
                # [128, W, 2] view of gate plane g over t in [a0, a0+W)
                return AP(tensor=gi_sb.tensor, offset=gi_sb.offset + 6 * a0 + g,
                          ap=[gi_sb.ap[0], [6, W], [3, 2]])

            # ---- production chunk ----
            def produce_chunk(c):
                x_sb = x_tiles.pop(c)
                git_pss = [
                    gtp.tile([3, 4 * B_LOC], F32, tag="git_ps", name="git_ps")
                    for _ in range(2)
                ]
                # cc outer: 4 consecutive matmuls share the same stationary
                # weights (fewer weight swaps -> fewer forced array drains)
                for cc in range(NCH):
                    for g in range(2):
                        for j in range(2):
                            nc.tensor.matmul(
                                git_pss[g][:, 512 * j:512 * (j + 1)],
                                wT_sb[:, cc, :],
                                x_sb[:, cc, 4 * g + 2 * j:4 * g + 2 * j + 2, :],
                                start=(cc == 0),
                                stop=(cc == NCH - 1),
                            )
                # one extra dummy matmul per chunk keeps the HAM window busy
                nc.tensor.matmul(dum_ps[:, 0:512], dum_w, dum_x,
                                 start=True, stop=True)
                git_sbs = []
                for g in range(2):
                    git_sb = gits.tile([3, 4 * B_LOC], BF16, name="git_sb")
                    nc.scalar.activation(
                        out=git_sb, in_=git_pss[g], func=AF.Identity,
                        bias=bias3, scale=1.0,
                    )
                    git_sbs.append(git_sb)
                gib = gbp.tile([128, 4 * TPC * NH], BF16, tag="gib", name="gib")
                for g in (1, 0):
                    for tg in range(4):
                        for h in range(NH):
                            tloc = 4 * g + tg
                            nc.tensor.transpose(
                                gib[:, 4 * (2 * tloc + h):4 * (2 * tloc + h) + 3],
                                git_sbs[g][:, 256 * tg + 128 * h:
                                           256 * tg + 128 * (h + 1)],
                                id3,
                            )
                # flush: gib cols 3*(2tl+h)+g -> gi_sb cols 6*(8c+tl)+3h+g
                for h in range(NH):
                    src = AP(tensor=gib.tensor, offset=gib.offset + 4 * h,
                             ap=[gib.ap[0], [8, TPC], [1, 3]])
                    dst = AP(tensor=gi_sb.tensor,
                             offset=gi_sb.offset + 6 * TPC * c + 3 * h,
                             ap=[gi_sb.ap[0], [6, TPC], [1, 3]])
                    nc.vector.tensor_copy(dst, src)

            # ---- Jacobi sweeps ----
            hbufs = {}

            def start_block(bi):
                s0, L, H, K = BLOCKS[bi]
                W = H + L
                hb = hbp.tile([128, 2 * W + 2], F32, tag="hb", name=f"hb{bi}")
                nc.gpsimd.memset(hb, 0.0)
                if H == 0:
                    nc.vector.tensor_copy(hb[:, 0:2], h0_sb)
                hbufs[bi] = hb

            def sweep(bi, k):
                s0, L, H, K = BLOCKS[bi]
                W = H + L
                a0 = s0 - H
                hb = hbufs[bi]
                hlag = hb[:, 0:2 * W]
                hl3 = hlag.rearrange("p (t h) -> p t h", h=2)
                # Engine split keeps every instruction at <=1 cross-engine
                # wait (single HW wait slot): DVE feeds ACT(sigmoid), Pool
                # feeds ACT(tanh); h-update closes on DVE.
                sigin = swp.tile([128, 4 * W], F32, tag="sigin", name="sigin")
                nc.vector.scalar_tensor_tensor(
                    sigin[:, 0:2 * W].rearrange("p (t h) -> p t h", h=2),
                    hl3, w0, gi_view(a0, W, 0), op0=ALU.mult, op1=ALU.add)
                nc.vector.scalar_tensor_tensor(
                    sigin[:, 2 * W:4 * W].rearrange("p (t h) -> p t h", h=2),
                    hl3, w1, gi_view(a0, W, 1), op0=ALU.mult, op1=ALU.add)
                rz = swp.tile([128, 4 * W], F32, tag="rz", name="rz")
                nc.scalar.activation(out=rz, in_=sigin, func=AF.Sigmoid)
                gh2 = swp.tile([128, 2 * W], F32, tag="gh2", name="gh2")
                nc.gpsimd.tensor_scalar(gh2, hlag, w2, bh2,
                                        op0=ALU.mult, op1=ALU.add)
                nr = swp.tile([128, 2 * W], F32, tag="nr", name="nr")
                nc.gpsimd.tensor_tensor(nr, rz[:, 0:2 * W], gh2, op=ALU.mult)
                npre = swp.tile([128, 2 * W], F32, tag="npre", name="npre")
                nc.gpsimd.tensor_tensor(
                    npre.rearrange("p (t h) -> p t h", h=2),
                    nr.rearrange("p (t h) -> p t h", h=2),
                    gi_view(a0, W, 2), op=ALU.add)
                zh = swp.tile([128, 2 * W], F32, tag="zh", name="zh")
                nc.vector.tensor_tensor(zh, rz[:, 2 * W:4 * W], hlag,
                                        op=ALU.mult)
                nt = swp.tile([128, 2 * W], F32, tag="nt", name="nt")
                nc.scalar.activation(out=nt, in_=npre, func=AF.Tanh)
                # nu = nt*(1-z) = nt - nt*z  (both inputs ACT-produced)
                m_t = swp.tile([128, 2 * W], F32, tag="m_t", name="m_t")
                nc.vector.tensor_tensor(m_t, nt, rz[:, 2 * W:4 * W],
                                        op=ALU.mult)
                nu = swp.tile([128, 2 * W], F32, tag="nu", name="nu")
                nc.vector.tensor_tensor(nu, nt, m_t, op=ALU.subtract)
                if k == K - 1:
                    # final sweep: write only the kept range, straight to hout
                    nc.vector.tensor_tensor(
                        hout[:, 2 * s0:2 * (s0 + L)], nu[:, 2 * H:2 * W],
                        zh[:, 2 * H:2 * W], op=ALU.add)
                else:
                    nc.vector.tensor_tensor(hb[:, 2:2 * W + 2], nu, zh,
                                            op=ALU.add)

            # ---- schedule: production chunks with interleaved sweeps ----
            ready_chunk = {
                bi: (s0 + L - 1) // TPC for bi, (s0, L, H, K) in enumerate(BLOCKS)
            }
            pending = []          # (bi, next_k) round-robin queue

            def emit_sweeps(n):
                cnt = 0
                while pending and cnt < n:
                    bi, k = pending.pop(0)
                    sweep(bi, k)
                    if k + 1 < BLOCKS[bi][3]:
                        pending.append((bi, k + 1))
                    cnt += 1

            for c in range(NCHUNK):
                if c + 3 < NCHUNK:
                    x_tiles[c + 3] = dma_chunk(c + 3)
                produce_chunk(c)
                for bi in range(len(BLOCKS)):
                    if ready_chunk[bi] == c:
                        start_block(bi)
                        pending.append((bi, 0))
                emit_sweeps(4 if c < 9 else 6)
            # drain remaining sweeps (round-robin keeps blocks interleaved)
            emit_sweeps(10 ** 9)

            # ---- batch-sum over partitions ----
            sum_ps = gtp.tile([1, 2 * T], F32, tag="git_ps", name="sum_ps")
            nc.tensor.matmul(sum_ps, ones_sb, hout, start=True, stop=True)
            sum_sb = scan.tile([1, 2 * T], F32)
            nc.vector.tensor_copy(sum_sb, sum_ps)
            nc.sync.dma_start(out=out[:], in_=sum_sb)

    _strip_same_engine_waits(nc)
    return nc


_ENG_PFX = {
    mybir.EngineType.Activation: "Activation",
    mybir.EngineType.DVE: "DVE",
    mybir.EngineType.PE: "PE",
    mybir.EngineType.Pool: "Pool",
    mybir.EngineType.SP: "SP",
}


def _strip_same_engine_waits(nc):
    """The compute-engine instruction formats have a single sync-wait slot.

    Tile's semaphore assignment is not transitively minimal and often adds a
    wait on the instruction's own engine semaphore next to a cross-engine
    wait. Engines execute their own stream in order, so same-engine waits
    are vacuous -- drop them when an instruction carries more than one wait.
    """
    multi = []
    for inst in nc.inst_map.values():
        si = inst.sync_info
        if not si or not si.on_wait or len(si.on_wait) <= 1:
            continue
        pfx = _ENG_PFX.get(inst.engine)
        if pfx is not None:
            kept = [
                w
                for w in si.on_wait
                if not (w.ant_name or "").startswith(pfx + "_")
            ]
            if len(kept) != len(si.on_wait):
                si.on_wait = kept
        if len(si.on_wait) > 1 and type(inst).__name__ == "InstDMACopy":
            comp = [
                w
                for w in si.on_wait
                if not (w.ant_name or "").startswith(("DMAHW", "DMASW"))
            ]
            if comp:
                si.on_wait = comp
        if len(si.on_wait) > 1:
            multi.append((inst.name, type(inst).__name__, str(inst.engine),
                          [w.ant_name for w in si.on_wait]))

    # Any instruction still carrying >1 wait cannot encode (single HW wait
    # slot): hoist all but one wait onto single-wait InstDrains inserted
    # just before it on the same engine.
    for block in nc.m.functions[0].blocks:
        insts = block.instructions
        for idx in range(len(insts) - 1, -1, -1):
            inst = insts[idx]
            si = inst.sync_info
            if not si or not si.on_wait or len(si.on_wait) <= 1:
                continue
            waits = list(si.on_wait)
            si.on_wait = waits[-1:]
            pre = []
            for k, w in enumerate(waits[:-1]):
                d = mybir.InstDrain(
                    name=f"{inst.name}-w{k}", ins=[], outs=[]
                )
                d.engine = inst.engine
                d.sync_info = mybir.SyncInfo(on_wait=[w], on_update=[])
                pre.append(d)
            insts[idx:idx] = pre
            multi = [m for m in multi if m[0] != inst.name]

    if multi:
        import sys
        print(f"[kernel] WARNING: {len(multi)} instructions still have >1 "
              f"sync wait: {multi[:8]}", file=sys.stderr)


def kernel(x, h0, w_ih, w_hh, b_ih, b_hh):
    import ml_dtypes
    bf16 = ml_dtypes.bfloat16

    x = np.asarray(x, dtype=np.float32)
    h0 = np.asarray(h0, dtype=np.float32)
    w_ih = np.asarray(w_ih, dtype=np.float32)
    w_hh = np.asarray(w_hh, dtype=np.float32)
    b_ih = np.asarray(b_ih, dtype=np.float32)
    b_hh = np.asarray(b_hh, dtype=np.float32)

    w0, w1, w2 = (float(v) for v in w_hh[:, 0])
    bh0, bh1, bh2 = (float(v) for v in b_hh)
    key = (w0, w1, w2, bh2)
    if _CACHE.get("key") != key:
        _CACHE["nc"] = build_nc(w0, w1, w2, bh2)
        _CACHE["key"] = key
    nc = _CACHE["nc"]

    wTb = np.ascontiguousarray(w_ih.T).astype(bf16)       # [D, 3]
    bias3 = np.array([b_ih[0] + bh0, b_ih[1] + bh1, b_ih[2]], dtype=np.float32)
    idb = np.zeros((3, 4), dtype=np.float32)
    idb[:, 0:3] = np.eye(3)
    idb = idb.astype(bf16)

    in_maps = []
    for c in range(N_CORES):
        xs = x[c * B_LOC:(c + 1) * B_LOC]                 # [B_loc, T, D]
        # [NCHUNK, D, TPC, B_loc]: 4KB-contiguous (t, b) runs per (chunk, d)
        xtb = np.ascontiguousarray(
            xs.reshape(B_LOC, NCHUNK, TPC, D).transpose(1, 3, 2, 0)
        ).astype(bf16)
        h0c = h0[0, c * B_LOC:(c + 1) * B_LOC, 0]         # [B_loc]
        h0t = h0c.reshape(NH, 128).T                      # [128, NH]
        cstc = np.zeros((128, 4), dtype=np.float32)
        cstc[:, 0:2] = h0t
        cstc[0:3, 2] = bias3
        in_maps.append({"xt": xtb, "wT": wTb, "cst": cstc, "idb": idb})

    res = run_bass_kernel_spmd(nc, in_maps, core_ids=list(range(N_CORES)))
    total = np.zeros((2 * T,), dtype=np.float64)
    for r in res.results:
        total += r["out"].reshape(-1).astype(np.float64)
    out = total.reshape(T, NH).sum(axis=1) / B
    return out.astype(np.float32)
